# revision 2
# baseline (speedup 1.0000x reference)
"""Trainium2 Bass kernel for MemoryEfficientAttnBlock3D — v2.

y = x + conv1x1(attn(conv1x1_{q,k,v}(groupnorm(x))), wp, bp)
x: (2, 64, 32, 32, 8) -> B=2, C=64, N=8192 tokens per batch.
8 cores = 2 batches x 4 query-chunks of 2048 tokens (rotated volumes,
identical program per core -- groupnorm stats and softmax reductions are
permutation-invariant over kv tokens).

v2 changes vs baseline (222us):
  - S matmuls: contraction is 64 (the bq ones-row is eliminated -- its
    per-kv-token score offset c_m = bq_eff^T k[:,m] moves into the exp
    bias, which is free). Two kv tiles run CONCURRENTLY as row-tiled
    matmuls (array rows 0-63 / 64-127) -> 2x S throughput. k lives in a
    split layout k2[128, 4096] (even kv tile -> partitions 0:64, odd ->
    64:128, pair p at cols 128p..128p+128); q is duplicated to both
    partition halves.
  - exp is split ACT/DVE: ACT pairs use the hardware Exp (-> fp8e4 out);
    DVE pairs use a Schraudolph bit-trick exp: one tensor_scalar
    t = round(a*s + b) -> uint8, whose bits ARE fp8e4 of e^(s+SHIFT).
    A global SHIFT=-2.7 keeps exp in fp8 range (softmax-invariant).
  - AV runs fp8 DoubleRow: one matmul contracts a 256-token kv pair
    (weights vt pairs [128,2,65] stride 80, rhs ex pairs [128,2,512]).
    v^T carries a ones column -> fused softmax denominator.
  - projections are col-tiled concurrent pairs (even/odd out partitions),
    single fp16 weights (no hi/lo -- fp8 ex dominates the error budget).
  - groupnorm normalize runs on GpSimd, stats split ACT/DVE, division
    tail uses reciprocal_approx_fast.
Measured numerics (host model): rel err ~4e-3 vs fp32 reference.
"""

import numpy as np

import concourse.bass as bass
import concourse.tile as tile
from concourse import bacc, mybir

F32 = mybir.dt.float32
F16 = mybir.dt.float16
F8 = mybir.dt.float8e4
U8 = mybir.dt.uint8
AF = mybir.ActivationFunctionType
OP = mybir.AluOpType

C = 64
GROUPS = 32
EPS = 1e-6

B_FULL = 2
H_FULL, W_FULL, D_FULL = 32, 32, 8
N_FULL = H_FULL * W_FULL * D_FULL  # 8192 kv tokens per batch
N_CORES = 8
Q_CHUNKS = 4
M_FULL = N_FULL // Q_CHUNKS  # 2048 q tokens per core

MB = 512      # q-token block
NT = 128      # kv tile (tokens)
PCH = 512     # production chunk (tokens) = 4 kv tiles = 2 pairs
VSTR = 160    # vt pair stride (fp8 bytes); sub A at +0:65, B at +80:145
LAG = 3       # pairs between S/exp and the consuming AV matmul

SHIFT = -2.7                   # score shift (softmax-invariant)
A_SCH = 8.0 / np.log(2.0)      # Schraudolph scale for e4m3
DELTA = -0.4                   # rounding-bias tweak
B_SCH = 56.0 + DELTA + A_SCH * SHIFT
ACT_EVERY = 7                  # pairs p with (p*4)//7 increment -> ACT
ACT_NUM = 4


def is_act_pair(p):
    return (p * ACT_NUM) // ACT_EVERY != ((p + 1) * ACT_NUM) // ACT_EVERY


def emit(tc, nc, n_tok, m_tok, xb_d, wq_d, wk_d, wpv_d, bpc_d, pair_d,
         expand_d, bq2_d, ct_bounce_d, out_d, has_c, dbg=None):
    ntiles = n_tok // NT          # 64
    npairs = ntiles // 2          # 32
    nch = 8
    sch = n_tok // nch            # 1024
    nchunks = n_tok // PCH        # 16

    with (
        tc.tile_pool(name="persist", bufs=1) as persist,
        tc.tile_pool(name="exS", bufs=5) as epool,
        tc.tile_pool(name="mtail", bufs=3) as mpool,
        tc.tile_pool(name="spsum", bufs=3, space="PSUM") as spool,
        tc.tile_pool(name="avpsum", bufs=1, space="PSUM") as avpool,
        tc.tile_pool(name="prodp", bufs=1, space="PSUM") as prodpool,
        tc.tile_pool(name="dram", bufs=2, space="DRAM") as dpool,
    ):
        xb_sb = persist.tile([C, n_tok], F32)
        xh_sb = persist.tile([C, n_tok], F16)
        k2_sb = persist.tile([128, n_tok // 2], F16)
        q2_sb = persist.tile([128, m_tok], F16)
        vt_sb = persist.tile([128, npairs * VSTR], F8)
        wq_sb = persist.tile([C, C], F16)
        wk_sb = persist.tile([C, C], F16)
        wpv_sb = persist.tile([C, C], F16)
        bpc_sb = persist.tile([C, 1], F32)
        pair_sb = persist.tile([C, GROUPS], F32)
        expand_sb = persist.tile([GROUPS, C], F32)
        stats_sb = persist.tile([C, 2 * nch], F32)
        scratch_sb = persist.tile([C, sch], F32)
        scratch2_sb = persist.tile([C, sch], F32)
        eps_sb = persist.tile([GROUPS, 1], F32)
        mrg_sb = persist.tile([GROUPS, 2], F32)
        mrc_sb = persist.tile([C, 2], F32)
        abias_sb = persist.tile([128, 1], F32)
        if has_c:
            bq2_sb = persist.tile([128, 2], F16)
            ct_sb = persist.tile([128, ntiles], F32)
            cbact_sb = persist.tile([128, ntiles], F32)
            cbdve_sb = persist.tile([128, ntiles], F32)
            csc_sb = persist.tile([2, n_tok // 2], F32)

        # ---- weight DMAs first (small; must not queue behind the 2MB xb) ----
        nc.sync.dma_start(out=wq_sb[:], in_=wq_d[:, :])
        nc.sync.dma_start(out=wk_sb[:], in_=wk_d[:, :])
        nc.sync.dma_start(out=wpv_sb[:], in_=wpv_d[:, :])
        nc.sync.dma_start(out=bpc_sb[:], in_=bpc_d[:, :])
        nc.sync.dma_start(out=pair_sb[:], in_=pair_d[:, :])
        nc.sync.dma_start(out=expand_sb[:], in_=expand_d[:, :])
        if has_c:
            nc.sync.dma_start(out=bq2_sb[:], in_=bq2_d[:, :])
        for ch in range(nch):
            sl = slice(ch * sch, (ch + 1) * sch)
            nc.sync.dma_start(out=xb_sb[:, sl], in_=xb_d[:, sl])
        nc.vector.memset(eps_sb[:], EPS)
        nc.vector.memset(abias_sb[:], SHIFT)
        # ones columns of v^T (fused softmax denominator)
        vt_view = vt_sb[:].rearrange("p (pr s) -> p pr s", s=VSTR)
        nc.gpsimd.memset(vt_view[:, :, C : C + 1], 1.0)
        nc.gpsimd.memset(vt_view[:, :, 80 + C : 80 + C + 1], 1.0)
        # zero a warmup zone of k2 so dummy matmuls read initialized data
        nc.vector.memset(k2_sb[:, 0:512], 0.0)

        # ---- PE warmup: dummy matmuls during the stats head (HAM ramp);
        # they use a spool bank so the stats matmul isn't WAW-serialized
        for i in range(8):
            warm = spool.tile([128, 2 * MB], F32, tag="s", name="warm")
            nc.tensor.matmul(warm[0:C, 0:512], wq_sb[:], k2_sb[0:C, 0:512],
                             start=True, stop=True)

        # ---- groupnorm stats: sum + sum-of-squares, engines alternate per
        # chunk so both track the input DMA instead of serializing
        for ch in range(nch):
            sl = slice(ch * sch, (ch + 1) * sch)
            if ch % 2 == 0:
                nc.vector.tensor_scalar(
                    out=scratch2_sb[:], in0=xb_sb[:, sl], scalar1=1.0,
                    scalar2=None, op0=OP.mult, op1=OP.add,
                    accum_out=stats_sb[:, ch : ch + 1],
                )
                nc.scalar.activation(
                    out=scratch_sb[:], in_=xb_sb[:, sl], func=AF.Square,
                    accum_out=stats_sb[:, nch + ch : nch + ch + 1],
                )
            else:
                nc.scalar.activation(
                    out=scratch_sb[:], in_=xb_sb[:, sl], func=AF.Copy,
                    accum_out=stats_sb[:, ch : ch + 1],
                )
                nc.vector.scalar_tensor_tensor(
                    out=scratch2_sb[:], in0=xb_sb[:, sl], scalar=1.0,
                    in1=xb_sb[:, sl], op0=OP.mult, op1=OP.mult,
                    accum_out=stats_sb[:, nch + ch : nch + ch + 1],
                )
        gp = prodpool.tile([GROUPS, 2 * nch], F32, tag="prod", name="gp")
        nc.tensor.matmul(gp[:], pair_sb[:], stats_sb[:], start=True, stop=True)
        gsum = mpool.tile([GROUPS, 2], F32, tag="gsum")
        nc.vector.tensor_reduce(
            out=gsum[:], in_=gp[:].rearrange("p (s c) -> p s c", s=2),
            axis=mybir.AxisListType.X, op=OP.add,
        )
        msq = mpool.tile([GROUPS, 1], F32, tag="msq")
        nc.vector.tensor_mul(msq[:], gsum[:, 0:1], gsum[:, 0:1])
        nc.vector.tensor_copy(mrg_sb[:, 0:1], gsum[:, 0:1])
        nc.vector.tensor_sub(mrg_sb[:, 1:2], gsum[:, 1:2], msq[:])
        # rstd = exp(-0.5*ln(var+eps)): Ln and Exp share one ACT table set,
        # so this avoids the two sqrt-set table loads (~3us)
        lnv = mpool.tile([GROUPS, 1], F32, tag="lnv", name="lnv")
        nc.scalar.activation(
            out=lnv[:], in_=mrg_sb[:, 1:2], func=AF.Ln, bias=eps_sb[:],
        )
        nc.scalar.activation(
            out=mrg_sb[:, 1:2], in_=lnv[:], func=AF.Exp, scale=-0.5,
        )
        ep = prodpool.tile([C, 2], F32, tag="prod", name="ep")
        nc.tensor.matmul(ep[:], expand_sb[:], mrg_sb[:], start=True, stop=True)
        nc.vector.tensor_copy(mrc_sb[:], ep[:])

        # ---- normalize (DVE), emitted lazily per 2048-token macro chunk ----
        XCH = 2048

        def emit_xh(ch):
            sl = slice(ch * XCH, (ch + 1) * XCH)
            nc.vector.tensor_scalar(
                out=xh_sb[:, sl], in0=xb_sb[:, sl],
                scalar1=mrc_sb[:, 0:1], scalar2=mrc_sb[:, 1:2],
                op0=OP.subtract, op1=OP.mult,
            )

        # ---- projections ----
        def proj_q(j):
            # head-only: use spool tiles (3 bufs) so consecutive chunks don't
            # WAR-stall on a single psum bank
            sl = slice(j * PCH, (j + 1) * PCH)
            qp = spool.tile([128, 2 * MB], F32, tag="s", name="qp")
            nc.tensor.matmul(qp[0:C, 0:PCH], wq_sb[:], xh_sb[:, sl], start=True, stop=True)
            nc.tensor.matmul(qp[C:128, 0:PCH], wq_sb[:], xh_sb[:, sl], start=True, stop=True)
            nc.vector.tensor_copy(q2_sb[:, sl], qp[:, 0:PCH])

        def proj_k(c):
            # 4 kv tiles -> 2 pair blocks of [128, 128]; even tile in rows
            # 0:64, odd in 64:128 (concurrent col-tiled matmul pairs)
            kp = prodpool.tile([128, 2 * NT], F32, tag="prod", name="kp")
            for t in range(4):
                j = 4 * c + t
                pr, h = t // 2, t % 2
                nc.tensor.matmul(
                    kp[64 * h : 64 * h + 64, 128 * pr : 128 * pr + 128],
                    wk_sb[:], xh_sb[:, j * NT : (j + 1) * NT],
                    start=True, stop=True,
                )
            nc.vector.tensor_copy(k2_sb[:, c * 2 * NT : (c + 1) * 2 * NT], kp[:])
            if has_c:
                csl = slice(c * 2 * NT, (c + 1) * 2 * NT)
                cp = prodpool.tile([2, 2 * NT], F32, tag="prod", name="cp")
                nc.tensor.matmul(cp[:], bq2_sb[:], k2_sb[:, csl],
                                 start=True, stop=True)
                nc.vector.tensor_copy(csc_sb[:, csl], cp[:])

        def proj_v(c):
            vp = prodpool.tile([128, 4 * C], F32, tag="prod", name="vp")
            for t in range(4):
                j = 4 * c + t
                nc.tensor.matmul(
                    vp[:, t * C : (t + 1) * C],
                    xh_sb[:, j * NT : (j + 1) * NT], wpv_sb[:],
                    start=True, stop=True,
                )
            # one strided copy: [128, (2 pairs, 2 tiles), 64] -> vt pair layout
            nc.vector.tensor_copy(
                vt_view.rearrange("p pr (two rest) -> p pr two rest", two=2)
                [:, 2 * c : 2 * c + 2, :, 0:C],
                vp[:].rearrange("p (a b m) -> p a b m", a=2, b=2),
            )

        emit_xh(0)
        for j in range(m_tok // PCH):
            proj_q(j)

        # lazy k/v production interleaved with block-0 attention; xh macro
        # chunks are normalized just-in-time before the chunk that needs them
        prod_step = [0]
        xh_done = [1]

        def produce_until(pair_limit):
            # chunk c supplies pairs 2c, 2c+1; steps alternate k/v per chunk
            while prod_step[0] < 2 * nchunks:
                step = prod_step[0]
                c, which = step // 2, step % 2
                if 2 * c > pair_limit + 1:
                    break
                xneed = (c * PCH) // XCH
                while xh_done[0] <= xneed and xh_done[0] < n_tok // XCH:
                    emit_xh(xh_done[0])
                    xh_done[0] += 1
                if which == 0:
                    proj_k(c)
                else:
                    proj_v(c)
                prod_step[0] += 1

        # ---- attention ----
        ex_f8_of = {}

        def emit_exp(p, sp):
            ex = epool.tile([128, 2 * MB], U8, tag="ex")
            exf8 = ex[:].bitcast(F8)
            if is_act_pair(p):
                if not has_c:
                    nc.scalar.activation(out=exf8, in_=sp[:], func=AF.Exp,
                                         bias=abias_sb[:])
                else:
                    for h in range(2):
                        nc.scalar.activation(
                            out=exf8[:, h * MB : (h + 1) * MB],
                            in_=sp[:, h * MB : (h + 1) * MB], func=AF.Exp,
                            bias=cbact_sb[:, 2 * p + h : 2 * p + h + 1],
                        )
            else:
                if not has_c:
                    nc.vector.tensor_scalar(
                        out=ex[:], in0=sp[:], scalar1=A_SCH, scalar2=B_SCH,
                        op0=OP.mult, op1=OP.add,
                    )
                else:
                    for h in range(2):
                        nc.vector.tensor_scalar(
                            out=ex[:, h * MB : (h + 1) * MB],
                            in0=sp[:, h * MB : (h + 1) * MB],
                            scalar1=A_SCH,
                            scalar2=cbdve_sb[:, 2 * p + h : 2 * p + h + 1],
                            op0=OP.mult, op1=OP.add,
                        )
            return ex

        def emit_av(p, ex, av):
            nc.tensor.matmul(
                av[:],
                vt_view[:, p, :].rearrange("p (two m) -> p two m", two=2)[:, :, 0 : C + 1],
                ex[:].bitcast(F8).rearrange("p (two n) -> p two n", two=2),
                start=(p == 0), stop=(p == npairs - 1),
                perf_mode=mybir.MatmulPerfMode.DoubleRow,
            )

        if has_c:
            # bounce csc [2, n/2] -> ct [128, ntiles] (transpose via DRAM),
            # then build per-tile exp biases
            produce_until(2 * npairs)  # need all k first in this path
            nc.sync.dma_start(out=ct_bounce_d[:, :], in_=csc_sb[:])
            nc.sync.dma_start(
                out=ct_sb[:].rearrange("p (j2 h) -> p j2 h", h=2),
                in_=ct_bounce_d[:, :].rearrange("h (j2 p) -> p j2 h", p=128),
            )
            nc.vector.tensor_scalar(
                out=cbact_sb[:], in0=ct_sb[:], scalar1=1.0, scalar2=SHIFT,
                op0=OP.mult, op1=OP.add,
            )
            nc.vector.tensor_scalar(
                out=cbdve_sb[:], in0=ct_sb[:], scalar1=A_SCH, scalar2=B_SCH,
                op0=OP.mult, op1=OP.add,
            )

        def make_tail1(av_sb):
            def tail1():
                # single-lane reciprocal straight off the denominator row; it
                # has no DMA dependency so it can never head-block the DVE
                # queue, and the deferral points give the rd/rb broadcast
                # DMAs ~8 pairs of latency headroom
                recip = mpool.tile([1, MB], F32, tag="recip", name="recip")
                nc.vector.reciprocal(recip[:], av_sb[C : C + 1, :])
                rd = dpool.tile([1, MB], F32, tag="rd", name="rd")
                nc.sync.dma_start(out=rd[:], in_=recip[:])
                rb = mpool.tile([C, MB], F32, tag="rb", name="rb")
                nc.sync.dma_start(out=rb[:], in_=rd[:].to_broadcast([C, MB]))
                return rb
            return tail1

        def make_tail2(av_sb, msl):
            def tail2(rb):
                t1 = mpool.tile([C, MB], F32, tag="t1", name="t1")
                nc.vector.tensor_mul(t1[:], av_sb[0:C, :], rb[:])
                outt = mpool.tile([C, MB], F32, tag="outt", name="outt")
                nc.vector.scalar_tensor_tensor(
                    out=outt[:], in0=t1[:], scalar=bpc_sb[:], in1=xb_sb[:, msl],
                    op0=OP.add, op1=OP.add,
                )
                nc.sync.dma_start(out=out_d[:, msl], in_=outt[:])
            return tail2

        tail_stage1 = None
        tail_stage2 = None
        for mb in range(m_tok // MB):
            msl = slice(mb * MB, (mb + 1) * MB)
            av = avpool.tile([C + 1, MB], F32, tag="av")
            pending = []
            for p in range(npairs):
                if mb == 0:
                    produce_until(p + 2)
                if p == 8 and tail_stage1 is not None:
                    tail_rb = tail_stage1()
                    tail_stage1 = None
                if p == 24 and tail_stage2 is not None:
                    tail_stage2(tail_rb)
                    tail_stage2 = None
                sp = spool.tile([128, 2 * MB], F32, tag="s")
                nc.tensor.matmul(
                    sp[:, 0:MB], k2_sb[0:C, p * NT : (p + 1) * NT],
                    q2_sb[0:C, msl], start=True, stop=True,
                    tile_position=(0, 0),
                )
                nc.tensor.matmul(
                    sp[:, MB : 2 * MB], k2_sb[C:128, p * NT : (p + 1) * NT],
                    q2_sb[C:128, msl], start=True, stop=True,
                    tile_position=(64, 0),
                )
                ex_t = emit_exp(p, sp)
                if dbg is not None and mb == 0 and p in (0, 2):
                    scopy = mpool.tile([128, 2 * MB], F32, tag=f"dbg{p}",
                                       name=f"dbgs{p}")
                    nc.vector.tensor_copy(scopy[:], sp[:])
                    nc.sync.dma_start(out=dbg[f"sp{p}"][:, :], in_=scopy[:])
                    ecopy = mpool.tile([128, 2 * MB], F32, tag=f"dbge{p}",
                                       name=f"dbge{p}")
                    nc.vector.tensor_copy(ecopy[:], ex_t[:])  # u8 bits -> f32
                    nc.sync.dma_start(out=dbg[f"ex{p}"][:, :], in_=ecopy[:])
                pending.append((p, ex_t))
                if len(pending) > LAG:
                    pp, ex = pending.pop(0)
                    emit_av(pp, ex, av)
            for pp, ex in pending:
                emit_av(pp, ex, av)
            av_sb = mpool.tile([C + 1, MB], F32, tag="avsb", name="av_sb")
            nc.vector.tensor_copy(av_sb[:], av[:])
            if dbg is not None and mb == 0:
                nc.sync.dma_start(out=dbg["av0"][:, :], in_=av_sb[:])
                nc.sync.dma_start(out=dbg["vtb"][:, :], in_=vt_sb[:].bitcast(U8))
            if tail_stage1 is not None:  # edge case: stages not yet consumed
                tail_rb = tail_stage1()
                tail_stage1 = None
            if tail_stage2 is not None:
                tail_stage2(tail_rb)
                tail_stage2 = None
            tail_stage1 = make_tail1(av_sb)
            tail_stage2 = make_tail2(av_sb, msl)
        # drain the last block's tail
        tail_rb = tail_stage1()
        tail_stage2(tail_rb)


def build_program(n_tok=N_FULL, m_tok=M_FULL, has_c=False, with_dbg=False):
    nc = bacc.Bacc("TRN2", target_bir_lowering=False, debug=False)
    xb_d = nc.dram_tensor("xb", [C, n_tok], F32, kind="ExternalInput")
    wq_d = nc.dram_tensor("wq16", [C, C], F16, kind="ExternalInput")
    wk_d = nc.dram_tensor("wk16", [C, C], F16, kind="ExternalInput")
    wpv_d = nc.dram_tensor("wpv16", [C, C], F16, kind="ExternalInput")
    bpc_d = nc.dram_tensor("bpc", [C, 1], F32, kind="ExternalInput")
    pair_d = nc.dram_tensor("pair", [C, GROUPS], F32, kind="ExternalInput")
    expand_d = nc.dram_tensor("expand", [GROUPS, C], F32, kind="ExternalInput")
    bq2_d = None
    ct_bounce_d = None
    if has_c:
        bq2_d = nc.dram_tensor("bq2", [128, 2], F16, kind="ExternalInput")
        ct_bounce_d = nc.dram_tensor("ctb", [2, n_tok // 2], F32, kind="Internal")
    out_d = nc.dram_tensor("out", [C, m_tok], F32, kind="ExternalOutput")
    dbg = None
    if with_dbg:
        dbg = {}
        for p in (0, 2):
            dbg[f"sp{p}"] = nc.dram_tensor(f"sp{p}", [128, 2 * MB], F32,
                                           kind="ExternalOutput").ap()
            dbg[f"ex{p}"] = nc.dram_tensor(f"ex{p}", [128, 2 * MB], F32,
                                           kind="ExternalOutput").ap()
        dbg["av0"] = nc.dram_tensor("av0", [C + 1, MB], F32,
                                    kind="ExternalOutput").ap()
        dbg["vtb"] = nc.dram_tensor("vtb", [128, (N_FULL // NT // 2) * VSTR],
                                    U8, kind="ExternalOutput").ap()
    with tile.TileContext(nc) as tc:
        emit(tc, nc, n_tok, m_tok, xb_d.ap(), wq_d.ap(), wk_d.ap(),
             wpv_d.ap(), bpc_d.ap(), pair_d.ap(), expand_d.ap(),
             bq2_d.ap() if has_c else None,
             ct_bounce_d.ap() if has_c else None,
             out_d.ap(), has_c, dbg=dbg)
    nc.compile()
    return nc


def prep_weights(gamma, beta, wq, bq, wk, bk, wv, bv, wp, bp, n_tok=N_FULL):
    f32 = np.float32
    gamma, beta = gamma.astype(f32), beta.astype(f32)
    scale = f32(1.0) / np.sqrt(f32(C)).astype(f32)
    wq_eff = (wq * gamma[None, :]) * scale
    bq_eff = (wq @ beta + bq) * scale
    wk_eff = wk * gamma[None, :]
    wv_eff = wv * gamma[None, :]
    bv_eff = wv @ beta + bv
    bp_eff = (bp + wp @ bv_eff).astype(f32)
    wpv_eff = (wp @ wv_eff).astype(f32)

    pair = np.zeros((C, GROUPS), f32)
    pair[np.arange(C), np.arange(C) // 2] = f32(1.0) / f32(2 * n_tok)
    expand = np.zeros((GROUPS, C), f32)
    expand[np.arange(C) // 2, np.arange(C)] = 1.0

    has_c = bool(np.any(bq_eff != 0))
    shared = {
        "wq16": np.ascontiguousarray(wq_eff.T, f32).astype(np.float16),
        "wk16": np.ascontiguousarray(wk_eff.T, f32).astype(np.float16),
        "wpv16": np.ascontiguousarray(wpv_eff.T, f32).astype(np.float16),
        "bpc": bp_eff.reshape(C, 1),
        "pair": pair,
        "expand": expand,
    }
    if has_c:
        bq2 = np.zeros((128, 2), np.float16)
        bq2[0:C, 0] = bq_eff.astype(np.float16)
        bq2[C:128, 1] = bq_eff.astype(np.float16)
        shared["bq2"] = bq2
    return shared, has_c


_PROGRAM_CACHE = {}


def _get_program(n_tok, m_tok, has_c):
    key = (n_tok, m_tok, has_c)
    if key not in _PROGRAM_CACHE:
        _PROGRAM_CACHE[key] = build_program(n_tok, m_tok, has_c)
    return _PROGRAM_CACHE[key]


def make_in_maps(x, shared):
    in_maps = []
    for core in range(N_CORES):
        b, qc = core // Q_CHUNKS, core % Q_CHUNKS
        xb = np.ascontiguousarray(x[b].reshape(C, N_FULL), np.float32)
        xb = np.ascontiguousarray(np.roll(xb, -qc * M_FULL, axis=1))
        in_maps.append({"xb": xb, **shared})
    return in_maps


def kernel(x, gamma, beta, wq, bq, wk, bk, wv, bv, wp, bp, **run_kwargs):
    from concourse.bass_utils import run_bass_kernel_spmd

    x = np.asarray(x, np.float32)
    shared, has_c = prep_weights(
        np.asarray(gamma), np.asarray(beta), np.asarray(wq), np.asarray(bq),
        np.asarray(wk), np.asarray(bk), np.asarray(wv), np.asarray(bv),
        np.asarray(wp), np.asarray(bp),
    )
    nc = _get_program(N_FULL, M_FULL, has_c)
    in_maps = make_in_maps(x, shared)
    res = run_bass_kernel_spmd(nc, in_maps, core_ids=list(range(N_CORES)), **run_kwargs)
    y = np.empty((B_FULL, C, N_FULL), np.float32)
    for core in range(N_CORES):
        b, qc = core // Q_CHUNKS, core % Q_CHUNKS
        y[b, :, qc * M_FULL : (qc + 1) * M_FULL] = res.results[core]["out"]
    out = y.reshape(B_FULL, C, H_FULL, W_FULL, D_FULL)
    if run_kwargs:
        return out, res
    return out
